# revision 16
# baseline (speedup 1.0000x reference)
"""MHA kernel for Trainium2, 8 NeuronCores.

Sharding: 4-way data parallel over batch x 2-way Megatron tensor parallel
over heads (8 heads / 512 dims per core). Wq/Wk/Wv split column-wise,
Wo split row-wise; the two TP partial outputs per batch are summed on host.

Per-core layout: everything runs in the "head-transposed" layout (Q^T, K^T
as [d, s]), so attention scores are computed as S^T = K Q^T with keys on
partitions, softmax denominators come from a ones-column appended to V, and
no transposes of S are needed.  RoPE uses a constant pair-rotation matrix
on the PE plus bf16 elementwise multiplies with precomputed cos/sin tables.

Scheduling: the PE frequency ramps 0.65 -> 1.2 -> 2.4 GHz and only holds
2.4 GHz while continuously busy, so the kernel is emitted as one long
gap-free PE stream: V-projection lead-in, then per head-pair the attention
inner loop (QK -> exp on ACT -> AV, scores double-buffered, AV lagged so
exp latency is hidden) with the NEXT pair's Q/K projection + RoPE (or the
output projection for the last pair) interleaved as filler matmuls.  Ready
work (AV + filler) is emitted ahead of the exp-dependent QKs each
iteration so semaphore latency is hidden.  Softmax denominators are
evacuated per 512-query block, inverted with a fast approximate
reciprocal, broadcast via a tiny 2-row matmul, and the normalize
multiplies run on the vector engine off the PE critical path.
"""

import math
import sys
from collections import deque

sys.path.insert(0, "/opt/trn_rl_repo")

import numpy as np

B = 4
S = 2048
DM = 1024
NH = 16
TP = 2                # tensor-parallel ways
HD = DM // TP         # 512 head-dims per core
NHL = NH // TP        # 8 local heads
DK = 64
NPAIR = NHL // 2      # 4 local head pairs (one 128-row chunk each)
P = 128
KCH = S // P          # 16 key chunks
NQT = 4               # 512-query blocks
QB = S // NQT         # 512
HB = 2 * QB           # 1024: x-chunk half width
NC_DM = DM // P       # 8 contraction chunks for projections
ROPE_THETA = 10000.0
SM_SCALE = 1.0 / math.sqrt(DK)
AV_LAG = 8            # AV(kc) is emitted AV_LAG iterations after QK(kc)

_CACHE = {}


def _build_nc():
    import concourse.bass as bass
    from concourse import mybir
    from concourse.tile import TileContext
    from contextlib import ExitStack

    f32 = mybir.dt.float32
    bf16 = mybir.dt.bfloat16
    EXP = mybir.ActivationFunctionType.Exp
    LN = mybir.ActivationFunctionType.Ln
    MUL = mybir.AluOpType.mult
    ADD = mybir.AluOpType.add

    nc = bass.Bass()
    xq = nc.declare_dram_parameter("xq_t", [DM, S], bf16, isOutput=False)
    xk = nc.declare_dram_parameter("xk_t", [DM, S], bf16, isOutput=False)
    xv = nc.declare_dram_parameter("xv_t", [DM, S], bf16, isOutput=False)
    wq = nc.declare_dram_parameter("wq", [DM, HD], bf16, isOutput=False)
    wk = nc.declare_dram_parameter("wk", [DM, HD], bf16, isOutput=False)
    wv = nc.declare_dram_parameter("wv", [DM, HD], bf16, isOutput=False)
    wo = nc.declare_dram_parameter("wo", [HD, DM], bf16, isOutput=False)
    cosd = nc.declare_dram_parameter("cos_t", [P, S], bf16, isOutput=False)
    sind = nc.declare_dram_parameter("sin_t", [P, S], bf16, isOutput=False)
    rotd = nc.declare_dram_parameter("rot_t", [P, P], bf16, isOutput=False)
    y = nc.declare_dram_parameter("y", [S, DM], f32, isOutput=True)

    with TileContext(nc) as tc, ExitStack() as top:
        persist = top.enter_context(tc.tile_pool(name="persist", bufs=1))
        vaug = persist.tile([P, KCH, NHL * 65], bf16)
        q_rot = persist.tile([P, NPAIR, S], bf16)
        k_rot = persist.tile([P, NPAIR, S], bf16)
        o_norm = persist.tile([P, NPAIR, S], bf16)
        wq_sb = persist.tile([P, NC_DM, HD], bf16)
        wk_sb = persist.tile([P, NC_DM, HD], bf16)
        wv_sb = persist.tile([P, NC_DM, HD], bf16)
        wo_sb = persist.tile([P, NPAIR, DM], bf16)
        cos_sb = persist.tile([P, S], bf16)
        sin_sb = persist.tile([P, S], bf16)
        rot_sb = persist.tile([P, P], bf16)
        ones1 = persist.tile([1, 64], bf16)
        warm = persist.tile([1, 2], f32)

        # rotating SBUF pools
        e_pool = top.enter_context(tc.tile_pool(name="e_pool", bufs=10))
        qb_pool = top.enter_context(tc.tile_pool(name="qb_pool", bufs=3))
        t_pool = top.enter_context(tc.tile_pool(name="t_pool", bufs=2))
        y_pool = top.enter_context(tc.tile_pool(name="y_pool", bufs=4))
        xb_pool = top.enter_context(tc.tile_pool(name="xb_pool", bufs=8))
        xvb_pool = top.enter_context(tc.tile_pool(name="xvb_pool", bufs=16))
        nrm = top.enter_context(tc.tile_pool(name="nrm", bufs=1))

        # ---------------- constant / weight / input DMAs ----------------
        # order matters: everything the lead-in needs first, finely
        # interleaved so the first V-projection chain starts ASAP.
        wvr = wv.rearrange("(c p) n -> p c n", p=P)
        xvch = {}
        for c in range(NC_DM):
            nc.sync.dma_start(out=wv_sb[:, c, :], in_=wvr[:, c, :])
            t = xvb_pool.tile([P, HB], bf16, name="xvch")
            nc.sync.dma_start(out=t, in_=xv[c * P:(c + 1) * P, 0:HB])
            xvch[(0, c)] = t
        wqr = wq.rearrange("(c p) n -> p c n", p=P)
        wkr = wk.rearrange("(c p) n -> p c n", p=P)
        nc.sync.dma_start(out=wq_sb[:, :, 0:P], in_=wqr[:, :, 0:P])
        nc.sync.dma_start(out=wk_sb[:, :, 0:P], in_=wkr[:, :, 0:P])
        nc.sync.dma_start(out=rot_sb, in_=rotd[:, :])
        nc.sync.dma_start(out=cos_sb, in_=cosd[:, :])
        nc.sync.dma_start(out=sin_sb, in_=sind[:, :])
        for c in range(NC_DM):
            t = xvb_pool.tile([P, HB], bf16, name="xvch")
            nc.sync.dma_start(out=t, in_=xv[c * P:(c + 1) * P, HB:2 * HB])
            xvch[(1, c)] = t
        nc.vector.memset(ones1, 1.0)
        for h in range(NHL):
            nc.vector.memset(vaug[:, :, 65 * h + 64: 65 * h + 65], 1.0)
        nc.vector.memset(warm, 0.0)
        # pre-load the Exp activation table so the first real exp is fast
        nc.scalar.activation(warm, warm, EXP, scale=1.0)

        def late_dmas():
            nc.sync.dma_start(out=wq_sb[:, :, P:HD], in_=wqr[:, :, P:HD])
            nc.sync.dma_start(out=wk_sb[:, :, P:HD], in_=wkr[:, :, P:HD])
            nc.sync.dma_start(
                out=wo_sb, in_=wo.rearrange("(c p) n -> p c n", p=P))

        # ---------------- filler machinery ----------------
        # filler items: ("mm", fn) costs one PE matmul; ("side", fn) is free
        filler = deque()

        def pump(n):
            popped = 0
            while filler and popped < n:
                kind, fn = filler.popleft()
                fn()
                if kind == "mm":
                    popped += 1

        def drain():
            while filler:
                filler.popleft()[1]()

        def proj_rope_items(p, psum_pool, evac_act):
            """Items projecting pair p's Q and K and applying RoPE.
            All tiles are allocated at emission time via holders."""
            items = []
            for xt, wt, dest in ((xq, wq_sb, q_rot), (xk, wk_sb, k_rot)):
                chks = {}   # (hf, c) -> sbuf [P, HB] tile

                def mkdma(hf, xt=xt, chks=chks):
                    def go():
                        for c in range(NC_DM):
                            t = xb_pool.tile([P, HB], bf16, name="xblk")
                            nc.sync.dma_start(
                                out=t,
                                in_=xt[c * P:(c + 1) * P,
                                       hf * HB:(hf + 1) * HB])
                            chks[(hf, c)] = t
                    return go

                items.append(("side", mkdma(0)))
                pending_rot = None
                for nt in range(NQT):
                    if nt == 1:
                        items.append(("side", mkdma(1)))
                    hf, col = nt // 2, (nt % 2) * QB
                    hold = {}

                    def mm(c, nt=nt, p=p, wt=wt, hold=hold, chks=chks,
                           hf=hf, col=col, psum_pool=psum_pool):
                        if c == 0:
                            hold["qp"] = psum_pool.tile(
                                [P, QB], f32, name="mm_ps", tag="mm",
                                bufs=None)
                        nc.tensor.matmul(
                            hold["qp"],
                            lhsT=wt[:, c, p * P:(p + 1) * P],
                            rhs=chks[(hf, c)][:, col:col + QB],
                            start=(c == 0),
                            stop=(c == NC_DM - 1),
                        )
                    mm_items = [("mm", lambda c=c, mm=mm: mm(c))
                                for c in range(NC_DM)]
                    # delay the previous tile's rot matmul behind this
                    # chain's first half so its PSUM evac has landed
                    if pending_rot is not None:
                        items.extend(mm_items[:4])
                        items.append(pending_rot)
                        items.extend(mm_items[4:])
                    else:
                        items.extend(mm_items)

                    def evac(hold=hold, evac_act=evac_act):
                        t = qb_pool.tile([P, QB], bf16, name="qp_bf")
                        if evac_act:
                            nc.scalar.copy(out=t, in_=hold["qp"])
                        else:
                            nc.vector.tensor_copy(out=t, in_=hold["qp"])
                        hold["qpb"] = t
                    items.append(("side", evac))

                    def rot(nt=nt, dest=dest, p=p, hold=hold,
                            psum_pool=psum_pool):
                        qpb = hold["qpb"]
                        rp = psum_pool.tile([P, QB], f32, name="mm_ps",
                                            tag="mm", bufs=None)
                        nc.tensor.matmul(rp, lhsT=rot_sb, rhs=qpb)
                        rpb = qb_pool.tile([P, QB], bf16, name="rp_bf")
                        nc.vector.tensor_copy(out=rpb, in_=rp)
                        ns = slice(nt * QB, (nt + 1) * QB)
                        t1 = t_pool.tile([P, QB], bf16, name="t1")
                        nc.vector.tensor_tensor(t1, qpb, cos_sb[:, ns], MUL)
                        t2 = t_pool.tile([P, QB], bf16, name="t2")
                        nc.vector.tensor_tensor(t2, rpb, sin_sb[:, ns], MUL)
                        nc.vector.tensor_tensor(dest[:, p, ns], t1, t2, ADD)
                    pending_rot = ("mm", rot)
                items.append(pending_rot)
            return items

        def outproj_items(qt, pools, evac_act=False):
            """Output projection for query block qt (contracts all pairs).
            `pools` is a list of (pool, tag) cycled per chain so the tail
            can alternate across more PSUM banks."""
            items = []
            ci = 0
            for qc in range(qt * (QB // P), (qt + 1) * (QB // P)):
                for nh in range(DM // QB):
                    hold = {}
                    pool, tag = pools[ci % len(pools)]
                    ci += 1

                    def mm(p, qc=qc, nh=nh, hold=hold, pool=pool, tag=tag):
                        if p == 0:
                            hold["yp"] = pool.tile(
                                [P, QB], f32, name="mm_ps", tag=tag,
                                bufs=None)
                        nc.tensor.matmul(
                            hold["yp"],
                            lhsT=o_norm[:, p, qc * P:(qc + 1) * P],
                            rhs=wo_sb[:, p, nh * QB:(nh + 1) * QB],
                            start=(p == 0),
                            stop=(p == NPAIR - 1),
                        )
                    for p in range(NPAIR):
                        items.append(("mm", lambda p=p, mm=mm: mm(p)))

                    def evac(qc=qc, nh=nh, hold=hold, evac_act=evac_act):
                        ys = y_pool.tile([P, QB], f32, name="ys")
                        if evac_act:
                            nc.scalar.copy(out=ys, in_=hold["yp"])
                        else:
                            nc.vector.tensor_copy(out=ys, in_=hold["yp"])
                        nc.sync.dma_start(
                            out=y[qc * P:(qc + 1) * P,
                                  nh * QB:(nh + 1) * QB],
                            in_=ys,
                        )
                    items.append(("side", evac))
            return items

        # ---------------- lead-in: V projection + pair-0 proj ----------------
        with tc.tile_pool(name="vproj_ps", bufs=4, space="PSUM") as vps_pool, \
                tc.tile_pool(name="p0_ps", bufs=2, space="PSUM") as p0_pool:
            filler.extend(proj_rope_items(0, p0_pool, evac_act=True))
            for sc in range(KCH):
                if sc == 4:
                    late_dmas()
                hf, col = sc // 8, (sc % 8) * P
                vps = vps_pool.tile([P, HD], f32, name="vps")
                for c in range(NC_DM):
                    nc.tensor.matmul(
                        vps,
                        lhsT=xvch[(hf, c)][:, col:col + P],
                        rhs=wv_sb[:, c, :],
                        start=(c == 0),
                        stop=(c == NC_DM - 1),
                    )
                nc.scalar.copy(
                    out=vaug[:, sc, :].rearrange(
                        "p (h e) -> p h e", e=65)[:, :, 0:64],
                    in_=vps.rearrange("p (h e) -> p h e", e=64),
                )
                if sc >= 2:
                    pump(5)
            drain()

        # ---------------- attention with interleaved filler ----------------
        with tc.tile_pool(name="atn_s", bufs=2, space="PSUM") as atn_s, \
                tc.tile_pool(name="atn_o", bufs=1, space="PSUM") as atn_o, \
                tc.tile_pool(name="atn_mm", bufs=2, space="PSUM") as atn_mm:

            pending_norm = None

            def emit_norm_tail(job):
                # PE broadcast of the reciprocals + DVE normalize multiplies.
                p_, qt_, oA_, oB_, rcbA_, rcbB_ = job
                qs_ = slice(qt_ * QB, (qt_ + 1) * QB)
                bc = atn_mm.tile([P, QB], f32, name="mm_ps", tag="mm",
                                 bufs=None)
                nc.tensor.matmul(bc[0:64, :], lhsT=ones1, rhs=rcbA_)
                nc.tensor.matmul(bc[64:128, :], lhsT=ones1, rhs=rcbB_)
                bcs = nrm.tile([P, QB], f32, name="bcs")
                nc.vector.tensor_copy(out=bcs, in_=bc)
                nc.vector.tensor_tensor(
                    o_norm[0:64, p_, qs_], oA_[0:64, :], bcs[0:64, :], MUL)
                nc.vector.tensor_tensor(
                    o_norm[64:128, p_, qs_], oB_[0:64, :], bcs[64:128, :], MUL)

            for p in range(NPAIR):
                hA, hB = 2 * p, 2 * p + 1
                if p + 1 < NPAIR:
                    filler.extend(proj_rope_items(p + 1, atn_mm,
                                                  evac_act=False))
                for qt in range(NQT):
                    qs = slice(qt * QB, (qt + 1) * QB)
                    oA = atn_o.tile([65, QB], f32, name="oA")
                    oB = atn_o.tile([65, QB], f32, name="oB")
                    e_stash = {}

                    def av_h(kc, head, o_t, e_stash=e_stash):
                        et = e_stash[kc]
                        nc.tensor.matmul(
                            o_t, lhsT=vaug[:, kc, 65 * head:65 * head + 65],
                            rhs=et[:, (head % 2) * QB:(head % 2 + 1) * QB],
                            start=(kc == 0), stop=(kc == KCH - 1))

                    def av(kc, oA=oA, oB=oB, hA=hA, hB=hB, e_stash=e_stash):
                        av_h(kc, hA, oA)
                        av_h(kc, hB, oB)
                        e_stash.pop(kc)

                    for kc in range(KCH):
                        # ready work first: filler + lagged AV hide the
                        # semaphore latency of the exp -> QK dependency.
                        # During pair 2 a little proj filler is held back so
                        # pair 3's first window (whose out-proj filler is not
                        # ready yet) is not starved.
                        if p == NPAIR - 2 and qt == NQT - 1 and kc >= 8:
                            pass
                        else:
                            pump(2 if p == NPAIR - 1 else 1)
                        if kc >= AV_LAG:
                            av(kc - AV_LAG)
                        if kc == 4 and pending_norm is not None:
                            emit_norm_tail(pending_norm)
                            pending_norm = None
                        kcs = slice(kc * P, (kc + 1) * P)
                        st = atn_s.tile([P, 2 * QB], f32, name="s")
                        nc.tensor.matmul(
                            st[:, 0:QB],
                            lhsT=k_rot[0:64, p, kcs],
                            rhs=q_rot[0:64, p, qs])
                        nc.tensor.matmul(
                            st[:, QB:2 * QB],
                            lhsT=k_rot[64:128, p, kcs],
                            rhs=q_rot[64:128, p, qs])
                        et = e_pool.tile([P, 2 * QB], bf16, name="e")
                        nc.scalar.activation(et, st, EXP, scale=SM_SCALE)
                        e_stash[kc] = et
                    # flush head A first so its reciprocal (on the ACT
                    # engine, via exp(-ln(den)) -- the DVE reciprocal is
                    # lane-bound and 5x slower) overlaps head B's flush
                    for kc in range(KCH - AV_LAG, KCH):
                        av_h(kc, hA, oA)
                    lgA = nrm.tile([1, QB], f32, name="lgA")
                    nc.scalar.activation(lgA, oA[64:65, :], LN)
                    rcbA = nrm.tile([1, QB], bf16, name="rcbA", bufs=2)
                    nc.scalar.activation(rcbA, lgA, EXP, scale=-1.0)
                    for kc in range(KCH - AV_LAG, KCH):
                        av_h(kc, hB, oB)
                        e_stash.pop(kc)
                    lgB = nrm.tile([1, QB], f32, name="lgB")
                    nc.scalar.activation(lgB, oB[64:65, :], LN)
                    rcbB = nrm.tile([1, QB], bf16, name="rcbB", bufs=2)
                    nc.scalar.activation(rcbB, lgB, EXP, scale=-1.0)
                    pending_norm = (p, qt, oA, oB, rcbA, rcbB)
                    pump(2)
                    if p == NPAIR - 1 and qt > 0:
                        # out-projection of the previous query block is the
                        # filler for the last pair
                        filler.extend(outproj_items(qt - 1, [(atn_mm, "mm")]))
                if p < NPAIR - 1:
                    drain()
            # tail: last normalize + remaining output projection (alternates
            # between the mm pool and the now-idle score banks, evacuated on
            # the now-idle ACT engine)
            emit_norm_tail(pending_norm)
            pending_norm = None
            drain()
            for it in outproj_items(NQT - 1, [(atn_mm, "mm"), (atn_s, "s")],
                                    evac_act=True):
                it[1]()

    return nc


_CTRL_OPS = {"Drain", "EventSemaphore", "Nop"}


def _patch_bir_waits(bs, ctrl_lim=1, other_lim=1):
    """Split instructions with more sync-waits than this walrus build's CTRL
    struct supports: excess waits move onto wait-only Drain instructions
    inserted immediately before (waiting earlier is always safe)."""
    import orjson

    j = orjson.loads(bs)
    for f in j["functions"]:
        for b in f["blocks"]:
            out = []
            for i in b["instructions"]:
                si = i.get("sync_info")
                w = si.get("on_wait", []) if si else []
                lim = ctrl_lim if i.get("opcode") in _CTRL_OPS else other_lim
                if len(w) > lim:
                    extra, keep = w[:-lim], w[-lim:]
                    for k, ww in enumerate(extra):
                        out.append({
                            "debug": i.get("debug", 0),
                            "engine": i["engine"],
                            "ins": [], "outs": [],
                            "is_reset_sema": False,
                            "name": i["name"] + f"-ws{k}",
                            "opcode": "Drain",
                            "sync_info": {"on_update": [], "on_wait": [ww]},
                        })
                    si["on_wait"] = keep
                out.append(i)
            b["instructions"] = out
    return orjson.dumps(j)


def _tables():
    if "tables" not in _CACHE:
        import ml_dtypes

        bf = ml_dtypes.bfloat16
        inv_freq = (ROPE_THETA ** (
            -np.arange(0, DK, 2, dtype=np.float32) / np.float32(DK)
        )).astype(np.float32)
        ang = np.arange(S, dtype=np.float32)[:, None] * inv_freq[None, :]
        cos_sj = np.cos(ang).astype(np.float32).T        # [32, S]
        sin_sj = np.sin(ang).astype(np.float32).T
        cos_t = np.tile(np.repeat(cos_sj, 2, axis=0), (2, 1))  # [128, S]
        sin_t = np.tile(np.repeat(sin_sj, 2, axis=0), (2, 1))
        prot = np.zeros((P, P), dtype=np.float32)
        for j in range(P // 2):
            prot[2 * j, 2 * j + 1] = -1.0
            prot[2 * j + 1, 2 * j] = 1.0
        rot_t = np.ascontiguousarray(prot.T)
        _CACHE["tables"] = (
            np.ascontiguousarray(cos_t).astype(bf),
            np.ascontiguousarray(sin_t).astype(bf),
            rot_t.astype(bf),
        )
    return _CACHE["tables"]


def _get_nc():
    if "nc" not in _CACHE:
        nc = _build_nc()
        orig = nc.to_json_bytes
        nc.to_json_bytes = lambda: _patch_bir_waits(orig())
        _CACHE["nc"] = nc
    return _CACHE["nc"]


def _run(in_maps, **kwargs):
    from concourse.bass_utils import run_bass_kernel_spmd

    return run_bass_kernel_spmd(_get_nc(), in_maps, core_ids=list(range(B * TP)),
                                **kwargs)


def _make_in_maps(q, k, v, Wq, Wk, Wv, Wo):
    import ml_dtypes

    cos_t, sin_t, rot_t = _tables()
    bf = ml_dtypes.bfloat16
    f = np.float32
    in_maps = []
    for core in range(B * TP):
        b, t = divmod(core, TP)
        cs = slice(t * HD, (t + 1) * HD)
        in_maps.append({
            "xq_t": np.ascontiguousarray(np.asarray(q[b], f).T).astype(bf),
            "xk_t": np.ascontiguousarray(np.asarray(k[b], f).T).astype(bf),
            "xv_t": np.ascontiguousarray(np.asarray(v[b], f).T).astype(bf),
            "wq": np.ascontiguousarray(np.asarray(Wq, f)[:, cs]).astype(bf),
            "wk": np.ascontiguousarray(np.asarray(Wk, f)[:, cs]).astype(bf),
            "wv": np.ascontiguousarray(np.asarray(Wv, f)[:, cs]).astype(bf),
            "wo": np.ascontiguousarray(np.asarray(Wo, f)[cs, :]).astype(bf),
            "cos_t": cos_t,
            "sin_t": sin_t,
            "rot_t": rot_t,
        })
    return in_maps


def kernel(q, k, v, Wq, Wk, Wv, Wo):
    res = _run(_make_in_maps(q, k, v, Wq, Wk, Wv, Wo))
    out = np.zeros((B, S, DM), np.float32)
    for core in range(B * TP):
        out[core // TP] += res.results[core]["y"]
    return out


# revision 17
# speedup vs baseline: 1.0191x; 1.0191x over previous
"""MHA kernel for Trainium2, 8 NeuronCores.

Sharding: 4-way data parallel over batch x 2-way Megatron tensor parallel
over heads (8 heads / 512 dims per core). Wq/Wk/Wv split column-wise,
Wo split row-wise; the two TP partial outputs per batch are summed on host.

Per-core layout: everything runs in the "head-transposed" layout (Q^T, K^T
as [d, s]), so attention scores are computed as S^T = K Q^T with keys on
partitions, softmax denominators come from a ones-column appended to V, and
no transposes of S are needed.  RoPE uses a constant pair-rotation matrix
on the PE plus bf16 elementwise multiplies with precomputed cos/sin tables.

Scheduling: the PE frequency ramps 0.65 -> 1.2 -> 2.4 GHz and only holds
2.4 GHz while continuously busy, so the kernel is emitted as one long
gap-free PE stream: V-projection lead-in, then per head-pair the attention
inner loop (QK -> exp on ACT -> AV, scores double-buffered, AV lagged so
exp latency is hidden) with the NEXT pair's Q/K projection + RoPE (or the
output projection for the last pair) interleaved as filler matmuls.  Ready
work (AV + filler) is emitted ahead of the exp-dependent QKs each
iteration so semaphore latency is hidden.  Softmax denominators are
evacuated per 512-query block, inverted with a fast approximate
reciprocal, broadcast via a tiny 2-row matmul, and the normalize
multiplies run on the vector engine off the PE critical path.
"""

import math
import sys
from collections import deque

sys.path.insert(0, "/opt/trn_rl_repo")

import numpy as np

B = 4
S = 2048
DM = 1024
NH = 16
TP = 2                # tensor-parallel ways
HD = DM // TP         # 512 head-dims per core
NHL = NH // TP        # 8 local heads
DK = 64
NPAIR = NHL // 2      # 4 local head pairs (one 128-row chunk each)
P = 128
KCH = S // P          # 16 key chunks
NQT = 4               # 512-query blocks
QB = S // NQT         # 512
HB = 2 * QB           # 1024: x-chunk half width
NC_DM = DM // P       # 8 contraction chunks for projections
ROPE_THETA = 10000.0
SM_SCALE = 1.0 / math.sqrt(DK)
AV_LAG = 8            # AV(kc) is emitted AV_LAG iterations after QK(kc)

_CACHE = {}


def _build_nc():
    import concourse.bass as bass
    from concourse import mybir
    from concourse.tile import TileContext
    from contextlib import ExitStack

    f32 = mybir.dt.float32
    bf16 = mybir.dt.bfloat16
    EXP = mybir.ActivationFunctionType.Exp
    LN = mybir.ActivationFunctionType.Ln
    MUL = mybir.AluOpType.mult
    ADD = mybir.AluOpType.add

    nc = bass.Bass()
    xq = nc.declare_dram_parameter("xq_t", [DM, S], bf16, isOutput=False)
    xk = nc.declare_dram_parameter("xk_t", [DM, S], bf16, isOutput=False)
    xv = nc.declare_dram_parameter("xv_t", [DM, S], bf16, isOutput=False)
    wq = nc.declare_dram_parameter("wq", [DM, HD], bf16, isOutput=False)
    wk = nc.declare_dram_parameter("wk", [DM, HD], bf16, isOutput=False)
    wv = nc.declare_dram_parameter("wv", [DM, HD], bf16, isOutput=False)
    wo = nc.declare_dram_parameter("wo", [HD, DM], bf16, isOutput=False)
    cosd = nc.declare_dram_parameter("cos_t", [P, S], bf16, isOutput=False)
    sind = nc.declare_dram_parameter("sin_t", [P, S], bf16, isOutput=False)
    rotd = nc.declare_dram_parameter("rot_t", [P, P], bf16, isOutput=False)
    y = nc.declare_dram_parameter("y", [S, DM], f32, isOutput=True)

    with TileContext(nc) as tc, ExitStack() as top:
        persist = top.enter_context(tc.tile_pool(name="persist", bufs=1))
        vaug = persist.tile([P, KCH, NHL * 65], bf16)
        q_rot = persist.tile([P, NPAIR, S], bf16)
        k_rot = persist.tile([P, NPAIR, S], bf16)
        o_norm = persist.tile([P, NPAIR, S], bf16)
        wq_sb = persist.tile([P, NC_DM, HD], bf16)
        wk_sb = persist.tile([P, NC_DM, HD], bf16)
        wv_sb = persist.tile([P, NC_DM, HD], bf16)
        wo_sb = persist.tile([P, NPAIR, DM], bf16)
        cos_sb = persist.tile([P, S], bf16)
        sin_sb = persist.tile([P, S], bf16)
        rot_sb = persist.tile([P, P], bf16)
        ones1 = persist.tile([1, 64], bf16)
        warm = persist.tile([1, 2], f32)

        # rotating SBUF pools
        e_pool = top.enter_context(tc.tile_pool(name="e_pool", bufs=13))
        qb_pool = top.enter_context(tc.tile_pool(name="qb_pool", bufs=3))
        t_pool = top.enter_context(tc.tile_pool(name="t_pool", bufs=2))
        y_pool = top.enter_context(tc.tile_pool(name="y_pool", bufs=4))
        xb_pool = top.enter_context(tc.tile_pool(name="xb_pool", bufs=8))
        xvb_pool = top.enter_context(tc.tile_pool(name="xvb_pool", bufs=16))
        nrm = top.enter_context(tc.tile_pool(name="nrm", bufs=1))

        # ---------------- constant / weight / input DMAs ----------------
        # order matters: everything the lead-in needs first, finely
        # interleaved so the first V-projection chain starts ASAP.
        wvr = wv.rearrange("(c p) n -> p c n", p=P)
        xvch = {}
        for c in range(NC_DM):
            nc.sync.dma_start(out=wv_sb[:, c, :], in_=wvr[:, c, :])
            t = xvb_pool.tile([P, HB], bf16, name="xvch")
            nc.sync.dma_start(out=t, in_=xv[c * P:(c + 1) * P, 0:HB])
            xvch[(0, c)] = t
        wqr = wq.rearrange("(c p) n -> p c n", p=P)
        wkr = wk.rearrange("(c p) n -> p c n", p=P)
        nc.sync.dma_start(out=wq_sb[:, :, 0:P], in_=wqr[:, :, 0:P])
        nc.sync.dma_start(out=wk_sb[:, :, 0:P], in_=wkr[:, :, 0:P])
        nc.sync.dma_start(out=rot_sb, in_=rotd[:, :])
        nc.sync.dma_start(out=cos_sb, in_=cosd[:, :])
        nc.sync.dma_start(out=sin_sb, in_=sind[:, :])
        for c in range(NC_DM):
            t = xvb_pool.tile([P, HB], bf16, name="xvch")
            nc.sync.dma_start(out=t, in_=xv[c * P:(c + 1) * P, HB:2 * HB])
            xvch[(1, c)] = t
        nc.vector.memset(ones1, 1.0)
        for h in range(NHL):
            nc.vector.memset(vaug[:, :, 65 * h + 64: 65 * h + 65], 1.0)
        nc.vector.memset(warm, 0.0)
        # pre-load the Exp activation table so the first real exp is fast
        nc.scalar.activation(warm, warm, EXP, scale=1.0)

        def late_dmas():
            nc.sync.dma_start(out=wq_sb[:, :, P:HD], in_=wqr[:, :, P:HD])
            nc.sync.dma_start(out=wk_sb[:, :, P:HD], in_=wkr[:, :, P:HD])
            nc.sync.dma_start(
                out=wo_sb, in_=wo.rearrange("(c p) n -> p c n", p=P))

        # ---------------- filler machinery ----------------
        # filler items: ("mm", fn) costs one PE matmul; ("side", fn) is free
        filler = deque()

        def pump(n):
            popped = 0
            while filler and popped < n:
                kind, fn = filler.popleft()
                fn()
                if kind == "mm":
                    popped += 1

        def drain():
            while filler:
                filler.popleft()[1]()

        def proj_rope_items(p, psum_pool, evac_act):
            """Items projecting pair p's Q and K and applying RoPE.
            All tiles are allocated at emission time via holders."""
            items = []
            for xt, wt, dest in ((xq, wq_sb, q_rot), (xk, wk_sb, k_rot)):
                chks = {}   # (hf, c) -> sbuf [P, HB] tile

                def mkdma(hf, xt=xt, chks=chks):
                    def go():
                        for c in range(NC_DM):
                            t = xb_pool.tile([P, HB], bf16, name="xblk")
                            nc.sync.dma_start(
                                out=t,
                                in_=xt[c * P:(c + 1) * P,
                                       hf * HB:(hf + 1) * HB])
                            chks[(hf, c)] = t
                    return go

                items.append(("side", mkdma(0)))
                pending_rot = None
                for nt in range(NQT):
                    if nt == 1:
                        items.append(("side", mkdma(1)))
                    hf, col = nt // 2, (nt % 2) * QB
                    hold = {}

                    def mm(c, nt=nt, p=p, wt=wt, hold=hold, chks=chks,
                           hf=hf, col=col, psum_pool=psum_pool):
                        if c == 0:
                            hold["qp"] = psum_pool.tile(
                                [P, QB], f32, name="mm_ps", tag="mm",
                                bufs=None)
                        nc.tensor.matmul(
                            hold["qp"],
                            lhsT=wt[:, c, p * P:(p + 1) * P],
                            rhs=chks[(hf, c)][:, col:col + QB],
                            start=(c == 0),
                            stop=(c == NC_DM - 1),
                        )
                    mm_items = [("mm", lambda c=c, mm=mm: mm(c))
                                for c in range(NC_DM)]
                    # delay the previous tile's rot matmul behind this
                    # chain's first half so its PSUM evac has landed
                    if pending_rot is not None:
                        items.extend(mm_items[:4])
                        items.append(pending_rot)
                        items.extend(mm_items[4:])
                    else:
                        items.extend(mm_items)

                    def evac(hold=hold, evac_act=evac_act):
                        t = qb_pool.tile([P, QB], bf16, name="qp_bf")
                        if evac_act:
                            nc.scalar.copy(out=t, in_=hold["qp"])
                        else:
                            nc.vector.tensor_copy(out=t, in_=hold["qp"])
                        hold["qpb"] = t
                    items.append(("side", evac))

                    def rot(nt=nt, dest=dest, p=p, hold=hold,
                            psum_pool=psum_pool):
                        qpb = hold["qpb"]
                        rp = psum_pool.tile([P, QB], f32, name="mm_ps",
                                            tag="mm", bufs=None)
                        nc.tensor.matmul(rp, lhsT=rot_sb, rhs=qpb)
                        rpb = qb_pool.tile([P, QB], bf16, name="rp_bf")
                        nc.vector.tensor_copy(out=rpb, in_=rp)
                        ns = slice(nt * QB, (nt + 1) * QB)
                        t1 = t_pool.tile([P, QB], bf16, name="t1")
                        nc.vector.tensor_tensor(t1, qpb, cos_sb[:, ns], MUL)
                        t2 = t_pool.tile([P, QB], bf16, name="t2")
                        nc.vector.tensor_tensor(t2, rpb, sin_sb[:, ns], MUL)
                        nc.vector.tensor_tensor(dest[:, p, ns], t1, t2, ADD)
                    pending_rot = ("mm", rot)
                items.append(pending_rot)
            return items

        def outproj_items(qt, pools, evac_act=False):
            """Output projection for query block qt (contracts all pairs).
            `pools` is a list of (pool, tag) cycled per chain so the tail
            can alternate across more PSUM banks."""
            items = []
            ci = 0
            for qc in range(qt * (QB // P), (qt + 1) * (QB // P)):
                for nh in range(DM // QB):
                    hold = {}
                    pool, tag = pools[ci % len(pools)]
                    ci += 1

                    def mm(p, qc=qc, nh=nh, hold=hold, pool=pool, tag=tag):
                        if p == 0:
                            hold["yp"] = pool.tile(
                                [P, QB], f32, name="mm_ps", tag=tag,
                                bufs=None)
                        nc.tensor.matmul(
                            hold["yp"],
                            lhsT=o_norm[:, p, qc * P:(qc + 1) * P],
                            rhs=wo_sb[:, p, nh * QB:(nh + 1) * QB],
                            start=(p == 0),
                            stop=(p == NPAIR - 1),
                        )
                    for p in range(NPAIR):
                        items.append(("mm", lambda p=p, mm=mm: mm(p)))

                    def evac(qc=qc, nh=nh, hold=hold, evac_act=evac_act):
                        ys = y_pool.tile([P, QB], f32, name="ys")
                        if evac_act:
                            nc.scalar.copy(out=ys, in_=hold["yp"])
                        else:
                            nc.vector.tensor_copy(out=ys, in_=hold["yp"])
                        nc.sync.dma_start(
                            out=y[qc * P:(qc + 1) * P,
                                  nh * QB:(nh + 1) * QB],
                            in_=ys,
                        )
                    items.append(("side", evac))
            return items

        # ---------------- lead-in: V projection + pair-0 proj ----------------
        with tc.tile_pool(name="vproj_ps", bufs=4, space="PSUM") as vps_pool, \
                tc.tile_pool(name="p0_ps", bufs=2, space="PSUM") as p0_pool:
            filler.extend(proj_rope_items(0, p0_pool, evac_act=True))
            for sc in range(KCH):
                if sc == 4:
                    late_dmas()
                hf, col = sc // 8, (sc % 8) * P
                vps = vps_pool.tile([P, HD], f32, name="vps")
                for c in range(NC_DM):
                    nc.tensor.matmul(
                        vps,
                        lhsT=xvch[(hf, c)][:, col:col + P],
                        rhs=wv_sb[:, c, :],
                        start=(c == 0),
                        stop=(c == NC_DM - 1),
                    )
                nc.scalar.copy(
                    out=vaug[:, sc, :].rearrange(
                        "p (h e) -> p h e", e=65)[:, :, 0:64],
                    in_=vps.rearrange("p (h e) -> p h e", e=64),
                )
                if sc >= 2:
                    pump(5)
            drain()

        # ---------------- attention with interleaved filler ----------------
        with tc.tile_pool(name="atn_s", bufs=2, space="PSUM") as atn_s, \
                tc.tile_pool(name="atn_o", bufs=1, space="PSUM") as atn_o, \
                tc.tile_pool(name="atn_mm", bufs=2, space="PSUM") as atn_mm:

            pending_norm = None

            def emit_norm_tail(job):
                # PE broadcast of the reciprocals + DVE normalize multiplies.
                p_, qt_, oA_, oB_, rcbA_, rcbB_ = job
                qs_ = slice(qt_ * QB, (qt_ + 1) * QB)
                bc = atn_mm.tile([P, QB], f32, name="mm_ps", tag="mm",
                                 bufs=None)
                nc.tensor.matmul(bc[0:64, :], lhsT=ones1, rhs=rcbA_)
                nc.tensor.matmul(bc[64:128, :], lhsT=ones1, rhs=rcbB_)
                bcs = nrm.tile([P, QB], f32, name="bcs")
                nc.vector.tensor_copy(out=bcs, in_=bc)
                nc.vector.tensor_tensor(
                    o_norm[0:64, p_, qs_], oA_[0:64, :], bcs[0:64, :], MUL)
                nc.vector.tensor_tensor(
                    o_norm[64:128, p_, qs_], oB_[0:64, :], bcs[64:128, :], MUL)

            for p in range(NPAIR):
                hA, hB = 2 * p, 2 * p + 1
                if p + 1 < NPAIR:
                    filler.extend(proj_rope_items(p + 1, atn_mm,
                                                  evac_act=False))
                for qt in range(NQT):
                    qs = slice(qt * QB, (qt + 1) * QB)
                    oA = atn_o.tile([65, QB], f32, name="oA")
                    oB = atn_o.tile([65, QB], f32, name="oB")
                    e_stash = {}

                    def av_h(kc, head, o_t, e_stash=e_stash):
                        et = e_stash[kc]
                        nc.tensor.matmul(
                            o_t, lhsT=vaug[:, kc, 65 * head:65 * head + 65],
                            rhs=et[:, (head % 2) * QB:(head % 2 + 1) * QB],
                            start=(kc == 0), stop=(kc == KCH - 1))

                    def av(kc, oA=oA, oB=oB, hA=hA, hB=hB, e_stash=e_stash):
                        av_h(kc, hA, oA)
                        av_h(kc, hB, oB)
                        e_stash.pop(kc)

                    av_done = 0
                    for kc in range(KCH):
                        # ready work first: filler + lagged AV hide the
                        # semaphore latency of the exp -> QK dependency.
                        # During pair 2 a little proj filler is held back so
                        # pair 3's first window (whose out-proj filler is not
                        # ready yet) is not starved.
                        if p == NPAIR - 2 and qt == NQT - 1 and kc >= 8:
                            pass
                        else:
                            pump(2 if p == NPAIR - 1 else 1)
                        # AVs start late (the o banks must wait for the
                        # previous window's normalize) then catch up 2/iter
                        if kc >= 11:
                            av(av_done)
                            av_done += 1
                            if av_done < 2 * (kc - 10):
                                av(av_done)
                                av_done += 1
                        if kc == 6 and pending_norm is not None:
                            emit_norm_tail(pending_norm)
                            pending_norm = None
                        kcs = slice(kc * P, (kc + 1) * P)
                        st = atn_s.tile([P, 2 * QB], f32, name="s")
                        nc.tensor.matmul(
                            st[:, 0:QB],
                            lhsT=k_rot[0:64, p, kcs],
                            rhs=q_rot[0:64, p, qs])
                        nc.tensor.matmul(
                            st[:, QB:2 * QB],
                            lhsT=k_rot[64:128, p, kcs],
                            rhs=q_rot[64:128, p, qs])
                        et = e_pool.tile([P, 2 * QB], bf16, name="e")
                        nc.scalar.activation(et, st, EXP, scale=SM_SCALE)
                        e_stash[kc] = et
                    # flush head A first so its (lane-bound, ~3.3us DVE)
                    # reciprocal overlaps head B's flush
                    for kc in range(av_done, KCH):
                        av_h(kc, hA, oA)
                    rcbA = nrm.tile([1, QB], bf16, name="rcbA", bufs=2)
                    with nc.allow_low_precision(reason="softmax recip bf16"):
                        nc.vector.reciprocal(rcbA, oA[64:65, :])
                    for kc in range(av_done, KCH):
                        av_h(kc, hB, oB)
                        e_stash.pop(kc)
                    rcbB = nrm.tile([1, QB], bf16, name="rcbB", bufs=2)
                    with nc.allow_low_precision(reason="softmax recip bf16"):
                        nc.vector.reciprocal(rcbB, oB[64:65, :])
                    pending_norm = (p, qt, oA, oB, rcbA, rcbB)
                    pump(2)
                    if p == NPAIR - 1 and qt > 0:
                        # out-projection of the previous query block is the
                        # filler for the last pair
                        filler.extend(outproj_items(qt - 1, [(atn_mm, "mm")]))
                if p < NPAIR - 1:
                    drain()
            # tail: last normalize + remaining output projection (alternates
            # between the mm pool and the now-idle score banks, evacuated on
            # the now-idle ACT engine)
            emit_norm_tail(pending_norm)
            pending_norm = None
            drain()
            for it in outproj_items(NQT - 1, [(atn_mm, "mm"), (atn_s, "s")],
                                    evac_act=True):
                it[1]()

    return nc


_CTRL_OPS = {"Drain", "EventSemaphore", "Nop"}


def _patch_bir_waits(bs, ctrl_lim=1, other_lim=1):
    """Split instructions with more sync-waits than this walrus build's CTRL
    struct supports: excess waits move onto wait-only Drain instructions
    inserted immediately before (waiting earlier is always safe)."""
    import orjson

    j = orjson.loads(bs)
    for f in j["functions"]:
        for b in f["blocks"]:
            out = []
            for i in b["instructions"]:
                si = i.get("sync_info")
                w = si.get("on_wait", []) if si else []
                lim = ctrl_lim if i.get("opcode") in _CTRL_OPS else other_lim
                if len(w) > lim:
                    extra, keep = w[:-lim], w[-lim:]
                    for k, ww in enumerate(extra):
                        out.append({
                            "debug": i.get("debug", 0),
                            "engine": i["engine"],
                            "ins": [], "outs": [],
                            "is_reset_sema": False,
                            "name": i["name"] + f"-ws{k}",
                            "opcode": "Drain",
                            "sync_info": {"on_update": [], "on_wait": [ww]},
                        })
                    si["on_wait"] = keep
                out.append(i)
            b["instructions"] = out
    return orjson.dumps(j)


def _tables():
    if "tables" not in _CACHE:
        import ml_dtypes

        bf = ml_dtypes.bfloat16
        inv_freq = (ROPE_THETA ** (
            -np.arange(0, DK, 2, dtype=np.float32) / np.float32(DK)
        )).astype(np.float32)
        ang = np.arange(S, dtype=np.float32)[:, None] * inv_freq[None, :]
        cos_sj = np.cos(ang).astype(np.float32).T        # [32, S]
        sin_sj = np.sin(ang).astype(np.float32).T
        cos_t = np.tile(np.repeat(cos_sj, 2, axis=0), (2, 1))  # [128, S]
        sin_t = np.tile(np.repeat(sin_sj, 2, axis=0), (2, 1))
        prot = np.zeros((P, P), dtype=np.float32)
        for j in range(P // 2):
            prot[2 * j, 2 * j + 1] = -1.0
            prot[2 * j + 1, 2 * j] = 1.0
        rot_t = np.ascontiguousarray(prot.T)
        _CACHE["tables"] = (
            np.ascontiguousarray(cos_t).astype(bf),
            np.ascontiguousarray(sin_t).astype(bf),
            rot_t.astype(bf),
        )
    return _CACHE["tables"]


def _get_nc():
    if "nc" not in _CACHE:
        nc = _build_nc()
        orig = nc.to_json_bytes
        nc.to_json_bytes = lambda: _patch_bir_waits(orig())
        _CACHE["nc"] = nc
    return _CACHE["nc"]


def _run(in_maps, **kwargs):
    from concourse.bass_utils import run_bass_kernel_spmd

    return run_bass_kernel_spmd(_get_nc(), in_maps, core_ids=list(range(B * TP)),
                                **kwargs)


def _make_in_maps(q, k, v, Wq, Wk, Wv, Wo):
    import ml_dtypes

    cos_t, sin_t, rot_t = _tables()
    bf = ml_dtypes.bfloat16
    f = np.float32
    in_maps = []
    for core in range(B * TP):
        b, t = divmod(core, TP)
        cs = slice(t * HD, (t + 1) * HD)
        in_maps.append({
            "xq_t": np.ascontiguousarray(np.asarray(q[b], f).T).astype(bf),
            "xk_t": np.ascontiguousarray(np.asarray(k[b], f).T).astype(bf),
            "xv_t": np.ascontiguousarray(np.asarray(v[b], f).T).astype(bf),
            "wq": np.ascontiguousarray(np.asarray(Wq, f)[:, cs]).astype(bf),
            "wk": np.ascontiguousarray(np.asarray(Wk, f)[:, cs]).astype(bf),
            "wv": np.ascontiguousarray(np.asarray(Wv, f)[:, cs]).astype(bf),
            "wo": np.ascontiguousarray(np.asarray(Wo, f)[cs, :]).astype(bf),
            "cos_t": cos_t,
            "sin_t": sin_t,
            "rot_t": rot_t,
        })
    return in_maps


def kernel(q, k, v, Wq, Wk, Wv, Wo):
    res = _run(_make_in_maps(q, k, v, Wq, Wk, Wv, Wo))
    out = np.zeros((B, S, DM), np.float32)
    for core in range(B * TP):
        out[core // TP] += res.results[core]["y"]
    return out


# revision 19
# speedup vs baseline: 1.1036x; 1.0829x over previous
"""MHA kernel for Trainium2, 8 NeuronCores.

Sharding: 4-way data parallel over batch x 2-way Megatron tensor parallel
over heads (8 heads / 512 dims per core). Wq/Wk/Wv split column-wise,
Wo split row-wise; the two TP partial outputs per batch are summed on host.

Per-core layout: everything runs in the "head-transposed" layout (Q^T, K^T
as [d, s]), so attention scores are computed as S^T = K Q^T with keys on
partitions, softmax denominators come from a ones-column appended to V, and
no transposes of S are needed.  RoPE uses a constant pair-rotation matrix
on the PE plus bf16 elementwise multiplies with precomputed cos/sin tables.

Scheduling: the PE frequency ramps 0.65 -> 1.2 -> 2.4 GHz and only holds
2.4 GHz while continuously busy, so the kernel is emitted as one long
gap-free PE stream: V-projection lead-in, then per head-pair the attention
inner loop (QK -> exp on ACT -> AV, scores double-buffered, AV lagged so
exp latency is hidden) with the NEXT pair's Q/K projection + RoPE (or the
output projection for the last pair) interleaved as filler matmuls.  Ready
work (AV + filler) is emitted ahead of the exp-dependent QKs each
iteration so semaphore latency is hidden.  Softmax denominators are
evacuated per 512-query block, inverted with a fast approximate
reciprocal, broadcast via a tiny 2-row matmul, and the normalize
multiplies run on the vector engine off the PE critical path.
"""

import math
import sys
from collections import deque

sys.path.insert(0, "/opt/trn_rl_repo")

import numpy as np

B = 4
S = 2048
DM = 1024
NH = 16
TP = 2                # tensor-parallel ways
HD = DM // TP         # 512 head-dims per core
NHL = NH // TP        # 8 local heads
DK = 64
NPAIR = NHL // 2      # 4 local head pairs (one 128-row chunk each)
P = 128
KCH = S // P          # 16 key chunks
NQT = 4               # 512-query blocks
QB = S // NQT         # 512
HB = 2 * QB           # 1024: x-chunk half width
NC_DM = DM // P       # 8 contraction chunks for projections
ROPE_THETA = 10000.0
SM_SCALE = 1.0 / math.sqrt(DK)
AV_LAG = 8            # AV(kc) is emitted AV_LAG iterations after QK(kc)

_CACHE = {}


def _build_nc():
    import concourse.bass as bass
    from concourse import mybir
    from concourse.tile import TileContext
    from contextlib import ExitStack

    f32 = mybir.dt.float32
    bf16 = mybir.dt.bfloat16
    EXP = mybir.ActivationFunctionType.Exp
    LN = mybir.ActivationFunctionType.Ln
    MUL = mybir.AluOpType.mult
    ADD = mybir.AluOpType.add

    nc = bass.Bass()
    xq = nc.declare_dram_parameter("xq_t", [DM, S], bf16, isOutput=False)
    xk = nc.declare_dram_parameter("xk_t", [DM, S], bf16, isOutput=False)
    xv = nc.declare_dram_parameter("xv_t", [DM, S], bf16, isOutput=False)
    wq = nc.declare_dram_parameter("wq", [DM, HD], bf16, isOutput=False)
    wk = nc.declare_dram_parameter("wk", [DM, HD], bf16, isOutput=False)
    wv = nc.declare_dram_parameter("wv", [DM, HD], bf16, isOutput=False)
    wo = nc.declare_dram_parameter("wo", [HD, DM], bf16, isOutput=False)
    cosd = nc.declare_dram_parameter("cos_t", [P, S], bf16, isOutput=False)
    sind = nc.declare_dram_parameter("sin_t", [P, S], bf16, isOutput=False)
    rotd = nc.declare_dram_parameter("rot_t", [P, P], bf16, isOutput=False)
    y = nc.declare_dram_parameter("y", [S, DM], f32, isOutput=True)

    with TileContext(nc) as tc, ExitStack() as top:
        persist = top.enter_context(tc.tile_pool(name="persist", bufs=1))
        vaug = persist.tile([P, KCH, NHL * 65], bf16)
        q_rot = persist.tile([P, NPAIR, S], bf16)
        k_rot = persist.tile([P, NPAIR, S], bf16)
        o_norm = persist.tile([P, NPAIR, S], bf16)
        wq_sb = persist.tile([P, NC_DM, HD], bf16)
        wk_sb = persist.tile([P, NC_DM, HD], bf16)
        wv_sb = persist.tile([P, NC_DM, HD], bf16)
        wo_sb = persist.tile([P, NPAIR, DM], bf16)
        cos_sb = persist.tile([P, S], bf16)
        sin_sb = persist.tile([P, S], bf16)
        rot_sb = persist.tile([P, P], bf16)
        ones1 = persist.tile([1, 64], bf16)
        warm = persist.tile([1, 2], f32)

        # rotating SBUF pools
        e_pool = top.enter_context(tc.tile_pool(name="e_pool", bufs=13))
        qb_pool = top.enter_context(tc.tile_pool(name="qb_pool", bufs=3))
        t_pool = top.enter_context(tc.tile_pool(name="t_pool", bufs=2))
        y_pool = top.enter_context(tc.tile_pool(name="y_pool", bufs=4))
        xb_pool = top.enter_context(tc.tile_pool(name="xb_pool", bufs=8))
        xvb_pool = top.enter_context(tc.tile_pool(name="xvb_pool", bufs=16))
        nrm = top.enter_context(tc.tile_pool(name="nrm", bufs=1))

        # ---------------- constant / weight / input DMAs ----------------
        # order matters: everything the lead-in needs first, finely
        # interleaved so the first V-projection chain starts ASAP.
        wvr = wv.rearrange("(c p) n -> p c n", p=P)
        xvch = {}
        for c in range(NC_DM):
            nc.sync.dma_start(out=wv_sb[:, c, :], in_=wvr[:, c, :])
            t = xvb_pool.tile([P, HB], bf16, name="xvch")
            nc.sync.dma_start(out=t, in_=xv[c * P:(c + 1) * P, 0:HB])
            xvch[(0, c)] = t
        wqr = wq.rearrange("(c p) n -> p c n", p=P)
        wkr = wk.rearrange("(c p) n -> p c n", p=P)
        nc.sync.dma_start(out=wq_sb[:, :, 0:P], in_=wqr[:, :, 0:P])
        nc.sync.dma_start(out=wk_sb[:, :, 0:P], in_=wkr[:, :, 0:P])
        nc.sync.dma_start(out=rot_sb, in_=rotd[:, :])
        nc.sync.dma_start(out=cos_sb, in_=cosd[:, :])
        nc.sync.dma_start(out=sin_sb, in_=sind[:, :])
        for c in range(NC_DM):
            t = xvb_pool.tile([P, HB], bf16, name="xvch")
            nc.sync.dma_start(out=t, in_=xv[c * P:(c + 1) * P, HB:2 * HB])
            xvch[(1, c)] = t
        nc.vector.memset(ones1, 1.0)
        for h in range(NHL):
            nc.vector.memset(vaug[:, :, 65 * h + 64: 65 * h + 65], 1.0)
        nc.vector.memset(warm, 0.0)
        # pre-load the Exp activation table so the first real exp is fast
        nc.scalar.activation(warm, warm, EXP, scale=1.0)

        def late_dmas():
            nc.sync.dma_start(out=wq_sb[:, :, P:HD], in_=wqr[:, :, P:HD])
            nc.sync.dma_start(out=wk_sb[:, :, P:HD], in_=wkr[:, :, P:HD])
            nc.sync.dma_start(
                out=wo_sb, in_=wo.rearrange("(c p) n -> p c n", p=P))

        # ---------------- filler machinery ----------------
        # filler items: ("mm", fn) costs one PE matmul; ("side", fn) is free
        filler = deque()

        def pump(n):
            popped = 0
            while filler and popped < n:
                kind, fn = filler.popleft()
                fn()
                if kind == "mm":
                    popped += 1

        def drain():
            while filler:
                filler.popleft()[1]()

        def proj_rope_items(p, psum_pool, evac_act):
            """Items projecting pair p's Q and K and applying RoPE.
            All tiles are allocated at emission time via holders."""
            items = []
            for xt, wt, dest in ((xq, wq_sb, q_rot), (xk, wk_sb, k_rot)):
                chks = {}   # (hf, c) -> sbuf [P, HB] tile

                def mkdma(hf, xt=xt, chks=chks):
                    def go():
                        for c in range(NC_DM):
                            t = xb_pool.tile([P, HB], bf16, name="xblk")
                            nc.sync.dma_start(
                                out=t,
                                in_=xt[c * P:(c + 1) * P,
                                       hf * HB:(hf + 1) * HB])
                            chks[(hf, c)] = t
                    return go

                items.append(("side", mkdma(0)))
                pending_rot = None
                for nt in range(NQT):
                    if nt == 1:
                        items.append(("side", mkdma(1)))
                    hf, col = nt // 2, (nt % 2) * QB
                    hold = {}

                    def mm(c, nt=nt, p=p, wt=wt, hold=hold, chks=chks,
                           hf=hf, col=col, psum_pool=psum_pool):
                        if c == 0:
                            hold["qp"] = psum_pool.tile(
                                [P, QB], f32, name="mm_ps", tag="mm",
                                bufs=None)
                        nc.tensor.matmul(
                            hold["qp"],
                            lhsT=wt[:, c, p * P:(p + 1) * P],
                            rhs=chks[(hf, c)][:, col:col + QB],
                            start=(c == 0),
                            stop=(c == NC_DM - 1),
                        )
                    mm_items = [("mm", lambda c=c, mm=mm: mm(c))
                                for c in range(NC_DM)]
                    # delay the previous tile's rot matmul behind this
                    # chain's first half so its PSUM evac has landed
                    if pending_rot is not None:
                        items.extend(mm_items[:4])
                        items.append(pending_rot)
                        items.extend(mm_items[4:])
                    else:
                        items.extend(mm_items)

                    def evac(hold=hold, evac_act=evac_act):
                        t = qb_pool.tile([P, QB], bf16, name="qp_bf")
                        if evac_act:
                            nc.scalar.copy(out=t, in_=hold["qp"])
                        else:
                            nc.vector.tensor_copy(out=t, in_=hold["qp"])
                        hold["qpb"] = t
                    items.append(("side", evac))

                    def rot(nt=nt, dest=dest, p=p, hold=hold,
                            psum_pool=psum_pool):
                        qpb = hold["qpb"]
                        rp = psum_pool.tile([P, QB], f32, name="mm_ps",
                                            tag="mm", bufs=None)
                        nc.tensor.matmul(rp, lhsT=rot_sb, rhs=qpb)
                        rpb = qb_pool.tile([P, QB], bf16, name="rp_bf")
                        nc.vector.tensor_copy(out=rpb, in_=rp)
                        ns = slice(nt * QB, (nt + 1) * QB)
                        t1 = t_pool.tile([P, QB], bf16, name="t1")
                        nc.vector.tensor_tensor(t1, qpb, cos_sb[:, ns], MUL)
                        t2 = t_pool.tile([P, QB], bf16, name="t2")
                        nc.vector.tensor_tensor(t2, rpb, sin_sb[:, ns], MUL)
                        nc.vector.tensor_tensor(dest[:, p, ns], t1, t2, ADD)
                    pending_rot = ("mm", rot)
                items.append(pending_rot)
            return items

        def outproj_items(qt, pools, evac_act=False):
            """Output projection for query block qt (contracts all pairs).
            `pools` is a list of (pool, tag) cycled per chain so the tail
            can alternate across more PSUM banks."""
            items = []
            ci = 0
            for qc in range(qt * (QB // P), (qt + 1) * (QB // P)):
                for nh in range(DM // QB):
                    hold = {}
                    pool, tag = pools[ci % len(pools)]
                    ci += 1

                    def mm(p, qc=qc, nh=nh, hold=hold, pool=pool, tag=tag):
                        if p == 0:
                            hold["yp"] = pool.tile(
                                [P, QB], f32, name="mm_ps", tag=tag,
                                bufs=None)
                        nc.tensor.matmul(
                            hold["yp"],
                            lhsT=o_norm[:, p, qc * P:(qc + 1) * P],
                            rhs=wo_sb[:, p, nh * QB:(nh + 1) * QB],
                            start=(p == 0),
                            stop=(p == NPAIR - 1),
                        )
                    for p in range(NPAIR):
                        items.append(("mm", lambda p=p, mm=mm: mm(p)))

                    def evac(qc=qc, nh=nh, hold=hold, evac_act=evac_act):
                        ys = y_pool.tile([P, QB], f32, name="ys")
                        if evac_act:
                            nc.scalar.copy(out=ys, in_=hold["yp"])
                        else:
                            nc.vector.tensor_copy(out=ys, in_=hold["yp"])
                        nc.sync.dma_start(
                            out=y[qc * P:(qc + 1) * P,
                                  nh * QB:(nh + 1) * QB],
                            in_=ys,
                        )
                    items.append(("side", evac))
            return items

        # ---------------- lead-in: V projection + pair-0 proj ----------------
        with tc.tile_pool(name="vproj_ps", bufs=4, space="PSUM") as vps_pool, \
                tc.tile_pool(name="p0_ps", bufs=2, space="PSUM") as p0_pool:
            filler.extend(proj_rope_items(0, p0_pool, evac_act=True))
            for sc in range(KCH):
                if sc == 4:
                    late_dmas()
                hf, col = sc // 8, (sc % 8) * P
                vps = vps_pool.tile([P, HD], f32, name="vps")
                for c in range(NC_DM):
                    nc.tensor.matmul(
                        vps,
                        lhsT=xvch[(hf, c)][:, col:col + P],
                        rhs=wv_sb[:, c, :],
                        start=(c == 0),
                        stop=(c == NC_DM - 1),
                    )
                nc.scalar.copy(
                    out=vaug[:, sc, :].rearrange(
                        "p (h e) -> p h e", e=65)[:, :, 0:64],
                    in_=vps.rearrange("p (h e) -> p h e", e=64),
                )
                if sc >= 2:
                    pump(5)
            drain()

        # ---------------- attention with interleaved filler ----------------
        with tc.tile_pool(name="atn_s", bufs=2, space="PSUM") as atn_s, \
                tc.tile_pool(name="atn_o", bufs=1, space="PSUM") as atn_o, \
                tc.tile_pool(name="atn_mm", bufs=2, space="PSUM") as atn_mm:

            pending_norm = None

            def emit_norm_tail(job):
                # PE broadcast of the reciprocals + DVE normalize multiplies.
                w = job
                qs_ = slice(w["qt"] * QB, (w["qt"] + 1) * QB)
                bc = atn_mm.tile([P, QB], f32, name="mm_ps", tag="mm",
                                 bufs=None)
                nc.tensor.matmul(bc[0:64, :], lhsT=ones1, rhs=w["rcbA"])
                nc.tensor.matmul(bc[64:128, :], lhsT=ones1, rhs=w["rcbB"])
                bcs = nrm.tile([P, QB], f32, name="bcs")
                nc.vector.tensor_copy(out=bcs, in_=bc)
                nc.vector.tensor_tensor(
                    o_norm[0:64, w["p"], qs_], w["oA"][0:64, :],
                    bcs[0:64, :], MUL)
                nc.vector.tensor_tensor(
                    o_norm[64:128, w["p"], qs_], w["oB"][0:64, :],
                    bcs[64:128, :], MUL)

            def av_h(w, head, kc):
                o_t = w["oA"] if head == 0 else w["oB"]
                h = 2 * w["p"] + head
                et = w["e_stash"][kc]
                nc.tensor.matmul(
                    o_t, lhsT=vaug[:, kc, 65 * h:65 * h + 65],
                    rhs=et[:, head * QB:(head + 1) * QB],
                    start=(kc == 0), stop=(kc == KCH - 1))

            def recip(w, head):
                rcb = nrm.tile([1, QB], bf16,
                               name="rcbA" if head == 0 else "rcbB", bufs=2)
                o_t = w["oA"] if head == 0 else w["oB"]
                with nc.allow_low_precision(reason="softmax recip bf16"):
                    nc.vector.reciprocal(rcb, o_t[64:65, :])
                w["rcbA" if head == 0 else "rcbB"] = rcb

            # trailing-AV schedule: the last 6 AV pairs of the previous
            # window run during this window's first iterations (their exps
            # are long done, so they never stall the PE); this window's own
            # AVs start at kc 12, after the previous normalize has freed
            # the o banks.
            CUR_AV = {12: (0, 1), 13: (2, 3), 14: (4, 5, 6), 15: (7, 8, 9)}

            def avsched(kc, win, prev):
                if prev is not None and kc <= 5:
                    if kc <= 2:
                        av_h(prev, 0, 10 + 2 * kc)
                        av_h(prev, 0, 11 + 2 * kc)
                        if kc == 2:
                            recip(prev, 0)
                    else:
                        av_h(prev, 1, 2 * kc + 4)
                        av_h(prev, 1, 2 * kc + 5)
                        if kc == 5:
                            recip(prev, 1)
                            for kc2 in range(10, KCH):
                                prev["e_stash"].pop(kc2)
                for kc2 in CUR_AV.get(kc, ()):
                    av_h(win, 0, kc2)
                    av_h(win, 1, kc2)
                    win["e_stash"].pop(kc2)

            prev_win = None
            for p in range(NPAIR):
                if p + 1 < NPAIR:
                    filler.extend(proj_rope_items(p + 1, atn_mm,
                                                  evac_act=False))
                for qt in range(NQT):
                    qs = slice(qt * QB, (qt + 1) * QB)
                    win = {
                        "p": p, "qt": qt, "e_stash": {},
                        "oA": atn_o.tile([65, QB], f32, name="oA"),
                        "oB": atn_o.tile([65, QB], f32, name="oB"),
                    }
                    for kc in range(KCH):
                        kcs = slice(kc * P, (kc + 1) * P)
                        st = atn_s.tile([P, 2 * QB], f32, name="s")
                        nc.tensor.matmul(
                            st[:, 0:QB],
                            lhsT=k_rot[0:64, p, kcs],
                            rhs=q_rot[0:64, p, qs])
                        nc.tensor.matmul(
                            st[:, QB:2 * QB],
                            lhsT=k_rot[64:128, p, kcs],
                            rhs=q_rot[64:128, p, qs])
                        et = e_pool.tile([P, 2 * QB], bf16, name="e")
                        nc.scalar.activation(et, st, EXP, scale=SM_SCALE)
                        win["e_stash"][kc] = et
                        avsched(kc, win, prev_win)
                        if kc == 9 and pending_norm is not None:
                            emit_norm_tail(pending_norm)
                            pending_norm = None
                        if kc == 5 and prev_win is not None:
                            pending_norm = prev_win
                            prev_win = None
                        # During pair 2 a little proj filler is held back so
                        # pair 3's first window (whose out-proj filler is not
                        # ready yet) is not starved.
                        if p == NPAIR - 2 and qt == NQT - 1 and kc >= 10:
                            pass
                        else:
                            pump(1)
                            if kc in (2, 5, 9) or p == NPAIR - 1:
                                pump(1)
                    prev_win = win
                    if p == NPAIR - 1 and qt > 0:
                        # out-projection of the previous query block is the
                        # filler for the last pair
                        filler.extend(outproj_items(qt - 1, [(atn_mm, "mm")]))
                if p < NPAIR - 1:
                    # pair p+1's projection filler must be fully emitted
                    # before its attention reads q_rot/k_rot
                    drain()
            # tail: trailing AVs + last normalize + remaining out-proj
            # (alternates between the mm pool and the now-idle score banks,
            # evacuated on the now-idle ACT engine)
            for kc in range(10, KCH):
                av_h(prev_win, 0, kc)
            recip(prev_win, 0)
            for kc in range(10, KCH):
                av_h(prev_win, 1, kc)
            recip(prev_win, 1)
            if pending_norm is not None:
                emit_norm_tail(pending_norm)
                pending_norm = None
            drain()
            emit_norm_tail(prev_win)
            for it in outproj_items(NQT - 1, [(atn_mm, "mm"), (atn_s, "s")],
                                    evac_act=True):
                it[1]()

    return nc


_CTRL_OPS = {"Drain", "EventSemaphore", "Nop"}


def _patch_bir_waits(bs, ctrl_lim=1, other_lim=1):
    """Split instructions with more sync-waits than this walrus build's CTRL
    struct supports: excess waits move onto wait-only Drain instructions
    inserted immediately before (waiting earlier is always safe)."""
    import orjson

    j = orjson.loads(bs)
    for f in j["functions"]:
        for b in f["blocks"]:
            out = []
            for i in b["instructions"]:
                si = i.get("sync_info")
                w = si.get("on_wait", []) if si else []
                lim = ctrl_lim if i.get("opcode") in _CTRL_OPS else other_lim
                if len(w) > lim:
                    extra, keep = w[:-lim], w[-lim:]
                    for k, ww in enumerate(extra):
                        out.append({
                            "debug": i.get("debug", 0),
                            "engine": i["engine"],
                            "ins": [], "outs": [],
                            "is_reset_sema": False,
                            "name": i["name"] + f"-ws{k}",
                            "opcode": "Drain",
                            "sync_info": {"on_update": [], "on_wait": [ww]},
                        })
                    si["on_wait"] = keep
                out.append(i)
            b["instructions"] = out
    return orjson.dumps(j)


def _tables():
    if "tables" not in _CACHE:
        import ml_dtypes

        bf = ml_dtypes.bfloat16
        inv_freq = (ROPE_THETA ** (
            -np.arange(0, DK, 2, dtype=np.float32) / np.float32(DK)
        )).astype(np.float32)
        ang = np.arange(S, dtype=np.float32)[:, None] * inv_freq[None, :]
        cos_sj = np.cos(ang).astype(np.float32).T        # [32, S]
        sin_sj = np.sin(ang).astype(np.float32).T
        cos_t = np.tile(np.repeat(cos_sj, 2, axis=0), (2, 1))  # [128, S]
        sin_t = np.tile(np.repeat(sin_sj, 2, axis=0), (2, 1))
        prot = np.zeros((P, P), dtype=np.float32)
        for j in range(P // 2):
            prot[2 * j, 2 * j + 1] = -1.0
            prot[2 * j + 1, 2 * j] = 1.0
        rot_t = np.ascontiguousarray(prot.T)
        _CACHE["tables"] = (
            np.ascontiguousarray(cos_t).astype(bf),
            np.ascontiguousarray(sin_t).astype(bf),
            rot_t.astype(bf),
        )
    return _CACHE["tables"]


def _get_nc():
    if "nc" not in _CACHE:
        nc = _build_nc()
        orig = nc.to_json_bytes
        nc.to_json_bytes = lambda: _patch_bir_waits(orig())
        _CACHE["nc"] = nc
    return _CACHE["nc"]


def _run(in_maps, **kwargs):
    from concourse.bass_utils import run_bass_kernel_spmd

    return run_bass_kernel_spmd(_get_nc(), in_maps, core_ids=list(range(B * TP)),
                                **kwargs)


def _make_in_maps(q, k, v, Wq, Wk, Wv, Wo):
    import ml_dtypes

    cos_t, sin_t, rot_t = _tables()
    bf = ml_dtypes.bfloat16
    f = np.float32
    in_maps = []
    for core in range(B * TP):
        b, t = divmod(core, TP)
        cs = slice(t * HD, (t + 1) * HD)
        in_maps.append({
            "xq_t": np.ascontiguousarray(np.asarray(q[b], f).T).astype(bf),
            "xk_t": np.ascontiguousarray(np.asarray(k[b], f).T).astype(bf),
            "xv_t": np.ascontiguousarray(np.asarray(v[b], f).T).astype(bf),
            "wq": np.ascontiguousarray(np.asarray(Wq, f)[:, cs]).astype(bf),
            "wk": np.ascontiguousarray(np.asarray(Wk, f)[:, cs]).astype(bf),
            "wv": np.ascontiguousarray(np.asarray(Wv, f)[:, cs]).astype(bf),
            "wo": np.ascontiguousarray(np.asarray(Wo, f)[cs, :]).astype(bf),
            "cos_t": cos_t,
            "sin_t": sin_t,
            "rot_t": rot_t,
        })
    return in_maps


def kernel(q, k, v, Wq, Wk, Wv, Wo):
    res = _run(_make_in_maps(q, k, v, Wq, Wk, Wv, Wo))
    out = np.zeros((B, S, DM), np.float32)
    for core in range(B * TP):
        out[core // TP] += res.results[core]["y"]
    return out
